# revision 32
# baseline (speedup 1.0000x reference)
"""ArcFace loss kernel for 8 TRN2 NeuronCores.

Strategy: tensor-parallel over classes (C=50000 -> 6250/core, padded to
6656 = 13*512).  Each core computes cos(emb, w_shard) with an fp8-e4m3
DoubleRow matmul (K=256 per instruction) and a fused exp+row-sum epilogue
on the scalar engine (per-row 1/||e|| folded into the activation scale).
Row/label norms and the label logit come from fp8 Gram-diagonal matmuls on
the transposed operands (cosine is scale-invariant, so fp8 scale factors
self-cancel).  A split AllReduce combines the per-core sum-exp vectors;
the margin-corrected log-softmax NLL mean is computed redundantly on every
core.

fp8 scaling: emb is cast raw (components ~N(0,1)); w rows are normalized
on-device and scaled x16 (the 1/16 folds into the exp scale); wlab is
scaled x64 (cancels in the cosine); squares for column norms are scaled
x65536 = 256^2 (the ln/exp norm chain subtracts the constant).
"""

import numpy as np

from concourse import bacc, bass, mybir, tile
from concourse import bass_utils
from concourse.bass_interp import get_hw_module
from concourse.masks import make_identity

B, D, C = 2048, 512, 50000
NCORES = 8
CS = C // NCORES            # 6250 classes per core
CSP = CS                    # no padding: ragged 106-wide tail chunks
PAD = 0
MARGIN = 0.3
SCALE = 30.0
EPS = 1e-12

F32 = mybir.dt.float32
BF16 = mybir.dt.bfloat16
FP8 = mybir.dt.float8e4
Act = mybir.ActivationFunctionType
Alu = mybir.AluOpType
DR = mybir.MatmulPerfMode.DoubleRow

NB = B // 128               # 16 batch tiles
NK = D // 128               # 4 contraction k-tiles (DR consumes pairs)
W8 = 16.0                   # fp8 scale on normalized weights
WL8 = 64.0                  # fp8 scale on label weights
SQ8 = 65536.0               # scale on squared raw weights (=256^2)
# main-loop column groups (1536-wide psum tiles)
JGROUPS = [(o, min(1536, CSP - o)) for o in range(0, CSP, 1536)]  # 4x1536 + 1x106
NJ = len(JGROUPS)           # 5
# weight-prep slabs
SLABS = [(0, 1024), (1024, 1024)] + [(o, min(2048, CSP - o)) for o in range(2048, CSP, 2048)]  # ragged tail



def _patch_act_tables():
    """Prefer natural_log_exp_and_others so alternating Ln/Exp activations
    resolve to one table set (avoids ~1.3us ACT_TABLE_LOAD thrash per switch)."""
    import concourse.hw_specs as hw_specs
    import concourse.bacc as bacc_mod
    orig = hw_specs.get_activation_tables
    def filtered(module_arch):
        tables = orig(module_arch)
        pref = "natural_log_exp_and_others"
        if pref in tables:
            # keep dict order/indices (act_func_set_id is positional) but
            # empty the competing exp/ln sets so the combined set is chosen
            tables = {
                k: (v if k == pref else {f for f in v
                                         if f not in tables[pref]})
                for k, v in tables.items()
            }
        return tables
    hw_specs.get_activation_tables = filtered
    bacc_mod.get_activation_tables = filtered


_patch_act_tables()

def build(stage="full"):
    nc = bacc.Bacc("TRN2", debug=False, num_devices=NCORES)

    embT_d = nc.dram_tensor("embT", [D, B], F32, kind="ExternalInput")
    wlabT_d = nc.dram_tensor("wlabT", [D, B], F32, kind="ExternalInput")
    wT_d = nc.dram_tensor("wT", [D, CSP], F32, kind="ExternalInput")
    out_d = nc.dram_tensor("out", [1, 1], F32, kind="ExternalOutput")

    with tile.TileContext(nc) as tc:
        with (
            tc.tile_pool(name="const", bufs=1) as constp,
            tc.tile_pool(name="res", bufs=1) as resp,
            tc.tile_pool(name="psum_cos", bufs=2, space="PSUM") as pcosp,
            tc.tile_pool(name="psum_aux", bufs=1, space="PSUM") as pauxp,
            tc.tile_pool(name="dram", bufs=1, space="DRAM") as dramp,
            tc.tile_pool(name="wraw", bufs=10) as wrawp,
            tc.tile_pool(name="prep", bufs=3) as prepp,
            tc.tile_pool(name="gscp", bufs=4) as gscp,
            tc.tile_pool(name="normp", bufs=4) as normp,
        ):
            ones8 = constp.tile([128, 2, 128], FP8, tag="ones8")
            nc.vector.memset(ones8[:], 1.0)
            ones_col = constp.tile([128, 1], F32, tag="ones_col")
            nc.vector.memset(ones_col[:], 1.0)
            ident = constp.tile([128, 128], F32, tag="ident")
            make_identity(nc, ident[:])
            # bias constant for the weight norm chain
            bias_w = constp.tile([128, 1], F32, tag="bias_w")
            nc.vector.memset(bias_w[:], float(np.log(W8) + 0.5 * np.log(SQ8)))

            # resident tensors
            ebT8 = resp.tile([128, NK, B], FP8, tag="ebT8")           # 8 KB/part
            wlT8 = resp.tile([128, NK, B], FP8, tag="wlT8")           # 8 KB/part
            wtn8 = resp.tile([128, NK, CSP], FP8, tag="wtn8")         # 26 KB/part
            Pcols = resp.tile([128, NB * NJ], F32, tag="Pcols")       # exp-sum accums
            sse_c = resp.tile([128, NB], F32, tag="sse_c")            # ||e8||^2
            ssw_c = resp.tile([128, NB], F32, tag="ssw_c")            # ||wl8||^2
            dot_c = resp.tile([128, NB], F32, tag="dot_c")            # e8 . wl8
            cosl_c = resp.tile([128, NB], F32, tag="cosl_c")          # cos at label
            s30_c = resp.tile([128, NB], F32, tag="s30_c")            # 30/(16||e||)
            inve_c = resp.tile([128, NB], F32, tag="inve_c")          # 1/||e8||


            def emit_ag(name, src_tile):
                cc_in = dramp.tile([128, NB], F32, name=f"agin_{name}")
                cc_out = dramp.tile([NCORES * 128, NB], F32, name=f"agout_{name}",
                                    addr_space="Shared")
                nc.gpsimd.dma_start(cc_in[:], src_tile[:])
                nc.gpsimd.collective_compute(
                    "AllGather", Alu.bypass,
                    replica_groups=[list(range(NCORES))],
                    ins=[cc_in[:].opt()], outs=[cc_out[:].opt()])
                return cc_out

            # warm-up collective: tiny AG so ncfw/SPAD is staged before the real ones
            warm_in = dramp.tile([128, 1], F32, name="warm_in")
            warm_out = dramp.tile([NCORES * 128, 1], F32, name="warm_out",
                                  addr_space="Shared")
            nc.gpsimd.dma_start(warm_in[:], ones_col[:])
            nc.gpsimd.collective_compute(
                "AllGather", Alu.bypass, replica_groups=[list(range(NCORES))],
                ins=[warm_in[:].opt()], outs=[warm_out[:].opt()])

            # ---- embT load + fp8 cast (gates both main matmul and exp scale) ----
            dma_engines = [nc.sync, nc.scalar, nc.gpsimd, nc.sync]
            for k in range(NK):
                et = wrawp.tile([128, 2048], F32, tag="wtraw")
                dma_engines[k].dma_start(et[:], embT_d.ap()[128 * k:128 * (k + 1), :])
                nc.vector.tensor_copy(ebT8[:, k, 0:1024], et[:, 0:1024])
                nc.vector.tensor_copy(ebT8[:, k, 1024:2048], et[:, 1024:2048])

            # ---- per-batch-tile row norms via fp8 Gram diagonal: gates exp ----
            # inve = exp(-0.5*ln(max(sse, EPS^2))); s30 = (SCALE/W8)*inve
            for i in range(NB):
                gps = pauxp.tile([128, 128], F32, tag="gram", bufs=1)
                for kk in range(NK // 2):
                    nc.tensor.matmul(
                        gps[:], ebT8[:, 2 * kk:2 * kk + 2, 128 * i:128 * (i + 1)],
                        ebT8[:, 2 * kk:2 * kk + 2, 128 * i:128 * (i + 1)],
                        start=(kk == 0), stop=(kk == NK // 2 - 1), perf_mode=DR)
                gsc = gscp.tile([128, 128], F32, tag="gsc")
                nc.vector.scalar_tensor_tensor(
                    gsc[:], gps[:], 1.0, ident[:], Alu.mult, Alu.mult,
                    accum_out=sse_c[:, i:i + 1])
                if i % 4 == 3:
                    b4 = slice(i - 3, i + 1)
                    nc.scalar.activation(inve_c[:, b4], sse_c[:, b4], Act.Ln)
                    nc.vector.tensor_scalar(
                        inve_c[:, b4], inve_c[:, b4], float(np.log(EPS * EPS)),
                        None, Alu.max)
                    nc.scalar.activation(inve_c[:, b4], inve_c[:, b4], Act.Exp,
                                         scale=-0.5)
                    nc.vector.tensor_scalar(
                        s30_c[:, b4], inve_c[:, b4], float(SCALE / W8),
                        None, Alu.mult)

            # ---- weight slabs: load, scaled squares, column norms, x16 fp8 ----
            # nv16 = W8/max(||w||, EPS) = exp(-0.5*ln(max(SQ8*ss, SQ8*EPS^2))
            #                                 + ln(W8) + 0.5*ln(SQ8))
            for (soff, ssz) in SLABS:
                wts = []
                wt28 = prepp.tile([128, NK, 2048], FP8, tag="wt28")
                for k in range(NK):
                    wt = wrawp.tile([128, 2048], F32, tag="wtraw")
                    nc.sync.dma_start(
                        wt[:, :ssz],
                        wT_d.ap()[128 * k:128 * (k + 1), soff:soff + ssz])
                    wts.append(wt)
                    nc.vector.scalar_tensor_tensor(
                        wt28[:, k, :ssz], wt[:, :ssz], float(SQ8), wt[:, :ssz],
                        Alu.mult, Alu.mult)
                for h0 in range(0, ssz, 512):
                    hsz = min(512, ssz - h0)
                    ss_ps = pauxp.tile([128, 512], F32, tag="ss", bufs=1)
                    for kk in range(NK // 2):
                        nc.tensor.matmul(
                            ss_ps[:, :hsz], ones8[:, :, :128],
                            wt28[:, 2 * kk:2 * kk + 2, h0:h0 + hsz],
                            start=(kk == 0), stop=(kk == NK // 2 - 1), perf_mode=DR)
                    nv = normp.tile([128, 512], F32, tag="nv")
                    nc.scalar.activation(nv[:, :hsz], ss_ps[:, :hsz], Act.Ln)
                    nc.vector.tensor_scalar(
                        nv[:, :hsz], nv[:, :hsz], float(np.log(SQ8 * EPS * EPS)),
                        None, Alu.max)
                    nc.scalar.activation(nv[:, :hsz], nv[:, :hsz], Act.Exp,
                                         scale=-0.5, bias=bias_w[:])
                    for k in range(NK):
                        nc.vector.tensor_mul(
                            wtn8[:, k, soff + h0:soff + h0 + hsz],
                            wts[k][:, h0:h0 + hsz], nv[:, :hsz])

            # ---- main loop: fp8 DR cos matmul + fused exp/row-sum ----
            ar_bufs = {}

            def emit_groups(jgroups, expop):
                for jji, (joff, jsz) in jgroups:
                    for i in range(NB):
                        ps = pcosp.tile([128, 1536], F32, tag="cos", name=f"ps{jji}_{i}")
                        for kk in range(NK // 2):
                            for h0 in range(0, jsz, 512):
                                hh = min(512, jsz - h0)
                                nc.tensor.matmul(
                                    ps[:, h0:h0 + hh],
                                    ebT8[:, 2 * kk:2 * kk + 2, 128 * i:128 * (i + 1)],
                                    wtn8[:, 2 * kk:2 * kk + 2, joff + h0:joff + h0 + hh],
                                    start=(kk == 0), stop=(kk == NK // 2 - 1),
                                    perf_mode=DR)
                        ex = expop.tile([128, 1536], BF16, tag="ex", name=f"ex{jji}_{i}")
                        nc.scalar.activation(
                            ex[:, :jsz], ps[:, :jsz], Act.Exp,
                            bias=0.0, scale=s30_c[:, i:i + 1],
                            accum_out=Pcols[:, i * NJ + jji:i * NJ + jji + 1])

            if stage != "prep":
                with tc.tile_pool(name="expo", bufs=4) as expop:
                    groups = list(enumerate(JGROUPS))
                    emit_groups(groups[:3], expop)
                    if stage == "full":
                        # AG#1 fires as soon as groups 0..2 are summed,
                        # overlapping groups 3..4 compute
                        P_a = resp.tile([128, NB], F32, tag="P_a")
                        nc.vector.tensor_reduce(
                            P_a[:],
                            Pcols[:].rearrange("p (i j) -> p i j", j=NJ)[:, :, 0:3],
                            mybir.AxisListType.X, Alu.add)
                        ar_bufs["a"] = emit_ag("a", P_a)
                    emit_groups(groups[3:], expop)

            # ---- late label path: wlabT norms + label dot (fp8 Gram diagonals) ----
            for k in range(NK):
                wlt = wrawp.tile([128, 2048], F32, tag="wtraw")
                nc.sync.dma_start(wlt[:], wlabT_d.ap()[128 * k:128 * (k + 1), :])
                nc.vector.tensor_scalar(wlT8[:, k, :], wlt[:], float(WL8), None, Alu.mult)
            for i in range(NB):
                gps2 = pauxp.tile([128, 256], F32, tag="gram", bufs=1)
                for kk in range(NK // 2):
                    nc.tensor.matmul(
                        gps2[:, 0:128], wlT8[:, 2 * kk:2 * kk + 2, 128 * i:128 * (i + 1)],
                        wlT8[:, 2 * kk:2 * kk + 2, 128 * i:128 * (i + 1)],
                        start=(kk == 0), stop=(kk == NK // 2 - 1), perf_mode=DR)
                for kk in range(NK // 2):
                    nc.tensor.matmul(
                        gps2[:, 128:256], ebT8[:, 2 * kk:2 * kk + 2, 128 * i:128 * (i + 1)],
                        wlT8[:, 2 * kk:2 * kk + 2, 128 * i:128 * (i + 1)],
                        start=(kk == 0), stop=(kk == NK // 2 - 1), perf_mode=DR)
                gsc2 = gscp.tile([128, 128], F32, tag="gsc")
                nc.vector.scalar_tensor_tensor(
                    gsc2[:], gps2[:, 0:128], 1.0, ident[:], Alu.mult, Alu.mult,
                    accum_out=ssw_c[:, i:i + 1])
                gsc3 = gscp.tile([128, 128], F32, tag="gsc")
                nc.vector.scalar_tensor_tensor(
                    gsc3[:], gps2[:, 128:256], 1.0, ident[:], Alu.mult, Alu.mult,
                    accum_out=dot_c[:, i:i + 1])

            # batched label math: invwl = 1/max(||wl8||, WL8*EPS) (scales cancel)
            invwl = resp.tile([128, NB], F32, tag="invwl")
            nc.scalar.activation(invwl[:], ssw_c[:], Act.Ln)
            nc.vector.tensor_scalar(
                invwl[:], invwl[:], float(np.log(WL8 * WL8 * EPS * EPS)), None, Alu.max)
            nc.scalar.activation(invwl[:], invwl[:], Act.Exp, scale=-0.5)
            nc.vector.tensor_mul(cosl_c[:], dot_c[:], inve_c[:])
            nc.vector.tensor_mul(cosl_c[:], cosl_c[:], invwl[:])

            # ---- split all-reduce + loss ----
            if stage == "full":
                with tc.tile_pool(name="fin", bufs=1) as finp:
                    cc_out_a = ar_bufs["a"]
                    # AG#2: groups 3..4
                    P_c = finp.tile([128, NB], F32, tag="P_c")
                    nc.vector.tensor_reduce(
                        P_c[:],
                        Pcols[:].rearrange("p (i j) -> p i j", j=NJ)[:, :, 3:NJ],
                        mybir.AxisListType.X, Alu.add)
                    cc_out_c = emit_ag("c", P_c)
                    # gather back: [8*128, NB] -> sbuf [128, 8, NB], reduce over cores
                    P_tot = finp.tile([128, NB], F32, tag="P_tot")
                    gs = []
                    for nm, cco in (("a", cc_out_a), ("c", cc_out_c)):
                        g = finp.tile([128, NCORES, NB], F32, tag=f"g_{nm}")
                        nc.gpsimd.dma_start(
                            g[:], cco[:].rearrange("(r p) j -> p r j", p=128))
                        gs.append(g)
                    red = finp.tile([128, 2, NB], F32, tag="red")
                    for gi, g in enumerate(gs):
                        nc.vector.tensor_reduce(
                            red[:, gi, :],
                            g[:].rearrange("p r j -> p j r"),
                            mybir.AxisListType.X, Alu.add)
                    nc.vector.tensor_reduce(
                        P_tot[:], red[:].rearrange("p t j -> p j t"),
                        mybir.AxisListType.X, Alu.add)

                    # margin: S = P_tot - npad - exp(30*cosl) + exp(30*cosl - 9)
                    e1 = finp.tile([128, NB], F32, tag="e1")
                    nc.scalar.activation(e1[:], cosl_c[:], Act.Exp,
                                         bias=0.0, scale=float(SCALE))
                    corr = finp.tile([128, NB], F32, tag="corr")
                    nc.vector.tensor_scalar(
                        corr[:], e1[:], float(np.exp(-MARGIN * SCALE) - 1.0),
                        None, Alu.mult)
                    S = finp.tile([128, NB], F32, tag="S")
                    nc.vector.tensor_add(S[:], P_tot[:], corr[:])
                    lnS = finp.tile([128, NB], F32, tag="lnS")
                    nc.scalar.activation(lnS[:], S[:], Act.Ln)
                    tgt = finp.tile([128, NB], F32, tag="tgt")
                    nc.vector.tensor_scalar(
                        tgt[:], cosl_c[:], float(SCALE), float(-MARGIN * SCALE),
                        Alu.mult, Alu.add)
                    nll = finp.tile([128, NB], F32, tag="nll")
                    nc.vector.tensor_sub(nll[:], lnS[:], tgt[:])
                    nrow = finp.tile([128, 1], F32, tag="nrow")
                    nc.vector.tensor_reduce(
                        nrow[:], nll[:], mybir.AxisListType.X, Alu.add)

                    ps11 = pauxp.tile([1, 1], F32, tag="gram", bufs=1,
                                      padded_shape=[1, 128])
                    nc.tensor.matmul(ps11[:], ones_col[:], nrow[:],
                                     start=True, stop=True)
                    loss_sb = finp.tile([1, 1], F32, tag="loss_sb")
                    nc.scalar.mul(loss_sb[:], ps11[:], 1.0 / B)
                    nc.sync.dma_start(out_d.ap()[:, :], loss_sb[:])

    nc.compile()
    nc.m = get_hw_module(nc.m)
    return nc


_NC_CACHE = None


def _get_nc():
    global _NC_CACHE
    if _NC_CACHE is None:
        import os
        _NC_CACHE = build(stage=os.environ.get("KERNEL_STAGE", "full"))
    return _NC_CACHE


def make_in_maps(embeddings, labels, weight):
    embeddings = np.ascontiguousarray(np.asarray(embeddings, dtype=np.float32))
    weight = np.ascontiguousarray(np.asarray(weight, dtype=np.float32))
    labels_i = np.asarray(labels).astype(np.int64)

    embT = np.ascontiguousarray(embeddings.T)
    wlabT = np.ascontiguousarray(weight[labels_i].T)

    in_maps = []
    for c in range(NCORES):
        shard = weight[c * CS:(c + 1) * CS]               # [6250, 512]
        wT = np.ascontiguousarray(shard.T)
        in_maps.append({"embT": embT, "wlabT": wlabT, "wT": wT})
    return in_maps


def kernel(embeddings, labels, weight, _trace=False, _trace_kwargs=None):
    in_maps = make_in_maps(embeddings, labels, weight)
    nc = _get_nc()
    res = bass_utils.run_bass_kernel_spmd(
        nc, in_maps, core_ids=list(range(NCORES)),
        trace=_trace, **(_trace_kwargs or {}))
    out = np.asarray(res.results[0]["out"], dtype=np.float32).reshape(())
    if _trace:
        kernel.last_result = res
    return out
